# revision 1
# baseline (speedup 1.0000x reference)
"""DKTForget Trainium2 kernel: embedding gather + one-hot c_integration +
LSTM scan + output projection, data-parallel over batch on 8 NeuronCores.

Self-contained: takes full (unsharded) inputs, shards batch across 8 cores,
runs one SPMD Bass kernel, gathers the full output.
"""
import os
import numpy as np
import ml_dtypes

import bass_rust
import concourse.bass as bass
import concourse.mybir as mybir
from concourse import library_config
from concourse.tile import TileContext, add_dep_helper
from concourse.bass_utils import run_bass_kernel_spmd

F32 = mybir.dt.float32
BF16 = mybir.dt.bfloat16
I16 = mybir.dt.int16
AF = mybir.ActivationFunctionType
ALU = mybir.AluOpType
BF = ml_dtypes.bfloat16

NUM_C = 2048
EMB = 256
NT = 64          # NUM_RGAP + NUM_SGAP + NUM_PCOUNT
H = 256
VOC = 2 * NUM_C  # 4096
B = 256
NCORES = 8
BLOC = B // NCORES  # 32 batch rows per core


def split_excess_waits(nc, maxw=1):
    """walrus CoreV3 allows a single sem-wait per instruction; split extras
    onto nop instructions inserted just before."""
    n_split = 0
    for f in nc.m.functions:
        for b in f.blocks:
            newlist = []
            for ins in b.instructions:
                si = ins.sync_info
                if si is not None and len(si.on_wait) > maxw:
                    w = list(si.on_wait)
                    k = 0
                    while len(w) - k > maxw:
                        nop = mybir.InstNoOp(name=f"{ins.name}_ws{k}", ins=[], outs=[])
                        nop.engine = ins.engine
                        nop.sync_info = bass_rust.SyncInfo(
                            on_wait=w[k:k + maxw], on_update=[])
                        newlist.append(nop)
                        k += maxw
                    ins.sync_info = bass_rust.SyncInfo(
                        on_wait=w[k:], on_update=list(si.on_update))
                    n_split += 1
                newlist.append(ins)
            b.instructions[:] = newlist
    return n_split


def build_nc(S=512, T_BLK=64):
    """Build the per-core Bass program (identical across cores)."""
    NTOK = S * BLOC
    NBLK = S // T_BLK
    BT = T_BLK * BLOC          # tokens per block
    GT = min(512, NTOK)        # phase-3 group tokens
    assert S % T_BLK == 0 and BT % 512 == 0 or BT % 128 == 0
    assert NTOK % GT == 0 and GT % 128 == 0

    nc = bass.Bass(trn_type="TRN2")

    # ---- I/O ----
    gidx = nc.dram_tensor("gidx", [128, NTOK // 16], I16, kind="ExternalInput")
    idxrows = nc.dram_tensor("idxrows", [7, NTOK], BF16, kind="ExternalInput")
    selpat = nc.dram_tensor("selpat", [7, 128], BF16, kind="ExternalInput")
    emb = nc.dram_tensor("emb", [VOC, EMB], BF16, kind="ExternalInput")
    wcembT = nc.dram_tensor("wcembT", [NT, EMB], BF16, kind="ExternalInput")
    wih = nc.dram_tensor("wih", [128, 16, 128], BF16, kind="ExternalInput")
    wihct = nc.dram_tensor("wihct", [NT, 8, 128], BF16, kind="ExternalInput")
    whh = nc.dram_tensor("whh", [128, 16, 128], BF16, kind="ExternalInput")
    bias8 = nc.dram_tensor("bias8", [128, 8], F32, kind="ExternalInput")
    ident = nc.dram_tensor("ident", [128, 128], BF16, kind="ExternalInput")
    woutT = nc.dram_tensor("woutT", [320, NUM_C], BF16, kind="ExternalInput")
    y = nc.dram_tensor("y", [NTOK, NUM_C], F32, kind="ExternalOutput")
    # internal DRAM spill for xg (precomputed input gates, transposed) and hT
    xgd = nc.dram_tensor("xgd", [128, 8, NTOK], BF16, kind="Internal")
    hTd = nc.dram_tensor("hTd", [128, 2, NTOK], BF16, kind="Internal")

    with TileContext(nc) as tc:
        with tc.tile_pool(name="const", bufs=1) as cpool, \
             tc.tile_pool(name="onehot", bufs=1) as ohpool:
            # ---- resident constants ----
            selpat_sb = cpool.tile([7, 128], BF16)
            nc.sync.dma_start(out=selpat_sb, in_=selpat[:, :])
            ident_sb = cpool.tile([128, 128], BF16)
            nc.sync.dma_start(out=ident_sb, in_=ident[:, :])
            wcembT_sb = cpool.tile([NT, EMB], BF16)
            nc.sync.dma_start(out=wcembT_sb, in_=wcembT[:, :])
            wih_sb = cpool.tile([128, 16 * 128], BF16)
            nc.sync.dma_start(out=wih_sb, in_=wih.rearrange("k t m -> k (t m)"))
            wihct_sb = cpool.tile([NT, 8 * 128], BF16)
            nc.sync.dma_start(out=wihct_sb, in_=wihct.rearrange("k t m -> k (t m)"))
            whh_sb = cpool.tile([128, 16 * 128], BF16)
            nc.sync.dma_start(out=whh_sb, in_=whh.rearrange("k t m -> k (t m)"))
            bias_sb = cpool.tile([128, 8], F32)
            nc.sync.dma_start(out=bias_sb, in_=bias8[:, :])
            w0_sb = cpool.tile([128, NUM_C], BF16)
            nc.sync.dma_start(out=w0_sb, in_=woutT[0:128, :])
            w1_sb = cpool.tile([128, NUM_C], BF16)
            nc.sync.dma_start(out=w1_sb, in_=woutT[128:256, :])
            w2_sb = cpool.tile([NT, NUM_C], BF16)
            nc.sync.dma_start(out=w2_sb, in_=woutT[256:320, :])
            gidx_sb = cpool.tile([128, NTOK // 16], I16)
            nc.sync.dma_start(out=gidx_sb, in_=gidx[:, :])

            # persistent one-hot^T buffers [NT, NTOK]
            oh_in = ohpool.tile([NT, NTOK], BF16)
            oh_out = ohpool.tile([NT, NTOK], BF16)

            # ---- P0: build one-hots for all tokens ----
            with tc.tile_pool(name="p0", bufs=2) as p0pool, \
                 tc.tile_pool(name="p0ps", bufs=2, space="PSUM") as p0ps:
                rows_sb = cpool.tile([7, NTOK], BF16)
                nc.sync.dma_start(out=rows_sb, in_=idxrows[:, :])
                CH = min(2048, NTOK)
                CW0 = min(512, CH)
                for ci in range(NTOK // CH):
                    sl = slice(ci * CH, (ci + 1) * CH)
                    for which, oh in ((0, oh_in), (1, oh_out)):
                        ps = p0ps.tile([NT, CH], F32, tag="idxps")
                        for c in range(CH // CW0):
                            nc.tensor.matmul(
                                ps[:, CW0 * c:CW0 * (c + 1)],
                                lhsT=selpat_sb[:, 64 * which:64 * which + 64],
                                rhs=rows_sb[:, ci * CH + CW0 * c: ci * CH + CW0 * (c + 1)],
                                start=True, stop=True)
                        nc.vector.tensor_scalar(
                            out=oh[:, sl], in0=ps, scalar1=0.0, scalar2=None,
                            op0=ALU.is_equal)

            # ---- P1: gather + theta_in + xg ----
            with tc.tile_pool(name="p1", bufs=2) as p1pool, \
                 tc.tile_pool(name="p1ps", bufs=2, space="PSUM") as p1ps:
                nc.gpsimd.load_library(library_config.mlp)
                CW = min(512, BT)
                for l in range(NBLK):
                    t0 = l * BT
                    xe = p1pool.tile([128, 2, BT], BF16, tag="xe")
                    nc.gpsimd.dma_gather(
                        xe, emb[:, :], gidx_sb[:, t0 // 16:(t0 + BT) // 16],
                        num_idxs=BT, num_idxs_reg=BT, elem_size=EMB,
                        transpose=True)
                    th = p1pool.tile([128, 2, BT], BF16, tag="th")
                    for e in range(2):
                        for c in range(BT // CW):
                            sl = slice(CW * c, CW * (c + 1))
                            gsl = slice(t0 + CW * c, t0 + CW * (c + 1))
                            psc = p1ps.tile([128, CW], F32, tag="cct")
                            nc.tensor.matmul(
                                psc, lhsT=wcembT_sb[:, 128 * e:128 * (e + 1)],
                                rhs=oh_in[:, gsl], start=True, stop=True)
                            nc.vector.tensor_mul(
                                out=th[:, e, sl], in0=xe[:, e, sl], in1=psc)
                    for j in range(8):
                        xs = p1pool.tile([128, BT], BF16, tag="xs")
                        for c in range(BT // CW):
                            sl = slice(CW * c, CW * (c + 1))
                            gsl = slice(t0 + CW * c, t0 + CW * (c + 1))
                            psx = p1ps.tile([128, CW], F32, tag="xg")
                            for k in range(3):
                                if k < 2:
                                    lhsT = wih_sb[:, (k * 8 + j) * 128:(k * 8 + j + 1) * 128]
                                    rhs = th[:, k, sl]
                                else:
                                    lhsT = wihct_sb[:, j * 128:(j + 1) * 128]
                                    rhs = oh_in[:, gsl]
                                nc.tensor.matmul(psx, lhsT=lhsT, rhs=rhs,
                                                 start=(k == 0), stop=(k == 2))
                            if (j + c) % 2 == 0:
                                nc.scalar.add(xs[:, sl], psx, bias_sb[:, j:j + 1])
                            else:
                                nc.vector.tensor_scalar(
                                    out=xs[:, sl], in0=psx,
                                    scalar1=bias_sb[:, j:j + 1], scalar2=None,
                                    op0=ALU.add)
                        nc.sync.dma_start(out=xgd[:, j, t0:t0 + BT], in_=xs)

            # ---- P2: LSTM scan ----
            with tc.tile_pool(name="p2", bufs=2) as p2pool, \
                 tc.tile_pool(name="p2s", bufs=3) as p2s, \
                 tc.tile_pool(name="p2ps", bufs=2, space="PSUM") as p2ps:
                hprev = None  # (tile, u) of previous step
                cprev = None
                last_mm = None  # previous step's final matmul (pin group order)
                for l in range(NBLK):
                    t0 = l * BT
                    xg_sb = p2pool.tile([128, 8, BT], BF16, tag="xgl")
                    nc.sync.dma_start(out=xg_sb, in_=xgd[:, :, t0:t0 + BT])
                    hb = p2pool.tile([128, 2, BT], BF16, tag="hb")
                    for u in range(T_BLK):
                        t = l * T_BLK + u
                        ps = p2ps.tile([128, 8, 32], F32, tag="gates")
                        mm = nc.tensor.matmul(
                            ps, lhsT=ident_sb, rhs=xg_sb[:, :, 32 * u:32 * (u + 1)],
                            start=True, stop=(t == 0), skip_group_check=True)
                        if last_mm is not None:
                            add_dep_helper(mm.ins, last_mm.ins,
                                           reason="keep psum groups contiguous")
                        last_mm = mm
                        if t > 0:
                            hbp, up = hprev
                            for j in range(8):
                                for k in range(2):
                                    last_mm = nc.tensor.matmul(
                                        ps[:, j, :],
                                        lhsT=whh_sb[:, (k * 8 + j) * 128:(k * 8 + j + 1) * 128],
                                        rhs=hbp[:, k, 32 * up:32 * (up + 1)],
                                        start=False, stop=(k == 1),
                                        skip_group_check=True)
                        sg = p2s.tile([128, 6, 32], BF16, tag="sg")
                        nc.scalar.activation(sg, ps[:, 0:6, :], AF.Sigmoid)
                        gg = p2s.tile([128, 2, 32], BF16, tag="gg")
                        nc.scalar.activation(gg, ps[:, 6:8, :], AF.Tanh)
                        cn = p2s.tile([128, 2, 32], F32, tag="cn")
                        if t == 0:
                            nc.vector.tensor_mul(cn, sg[:, 0:2, :], gg)
                        else:
                            m = p2s.tile([128, 2, 32], F32, tag="m")
                            nc.vector.tensor_mul(m, sg[:, 0:2, :], gg)
                            cf = p2s.tile([128, 2, 32], F32, tag="cf")
                            nc.vector.tensor_mul(cf, cprev, sg[:, 2:4, :])
                            nc.vector.tensor_add(cn, cf, m)
                        tct = p2s.tile([128, 2, 32], BF16, tag="tct")
                        nc.scalar.activation(tct, cn, AF.Tanh)
                        nc.vector.tensor_mul(
                            hb[:, :, 32 * u:32 * (u + 1)], sg[:, 4:6, :], tct)
                        hprev = (hb, u)
                        cprev = cn
                    nc.sync.dma_start(out=hTd[:, :, t0:t0 + BT], in_=hb)

            # ---- P3: theta_out + output matmul + sigmoid ----
            with tc.tile_pool(name="p3", bufs=2) as p3pool, \
                 tc.tile_pool(name="p3y", bufs=2) as p3y, \
                 tc.tile_pool(name="p3ps", bufs=2, space="PSUM") as p3ps, \
                 tc.tile_pool(name="p3yps", bufs=4, space="PSUM") as p3yps:
                GT_ = min(GT, BT)
                for l in range(NBLK):
                    t0 = l * BT
                    h3 = p3pool.tile([128, 2, BT], BF16, tag="h3")
                    nc.sync.dma_start(out=h3, in_=hTd[:, :, t0:t0 + BT])
                    for gi in range(BT // GT_):
                        g0 = t0 + gi * GT_
                        tho = p3pool.tile([128, 2, GT_], BF16, tag="tho")
                        for e in range(2):
                            psc = p3ps.tile([128, GT_], F32, tag="cct3")
                            w = min(512, GT_)
                            for c in range(GT_ // w):
                                nc.tensor.matmul(
                                    psc[:, w * c:w * (c + 1)],
                                    lhsT=wcembT_sb[:, 128 * e:128 * (e + 1)],
                                    rhs=oh_out[:, g0 + w * c:g0 + w * (c + 1)],
                                    start=True, stop=True)
                            nc.vector.tensor_mul(
                                out=tho[:, e, :],
                                in0=h3[:, e, gi * GT_:(gi + 1) * GT_], in1=psc)
                        for sub in range(GT_ // 128):
                            s0 = g0 + 128 * sub
                            ys = p3y.tile([128, NUM_C], F32, tag="ys")
                            for c in range(NUM_C // 512):
                                sl = slice(512 * c, 512 * (c + 1))
                                psy = p3yps.tile([128, 512], F32, tag="psy")
                                for k in range(3):
                                    if k < 2:
                                        lhsT = tho[:, k, 128 * sub:128 * (sub + 1)]
                                    else:
                                        lhsT = oh_out[:, s0:s0 + 128]
                                    rhs = (w0_sb, w1_sb, w2_sb)[k][:, sl]
                                    nc.tensor.matmul(psy, lhsT=lhsT, rhs=rhs,
                                                     start=(k == 0), stop=(k == 2))
                                nc.scalar.activation(ys[:, sl], psy, AF.Sigmoid)
                            nc.sync.dma_start(out=y[s0:s0 + 128, :], in_=ys)

    return nc


def build_nc_v2(S=512, T_BLK=64):
    """Interleaved single-pass build: phase-1 (gather/theta/xg for block l+1)
    and phase-3 (output projection for finished steps) are woven between the
    LSTM steps of block l, so PE/ACT stall windows of the serial recurrence do
    the bulk work. No DRAM spills: xg and h stay in SBUF."""
    NTOK = S * BLOC
    NBLK = S // T_BLK
    BT = T_BLK * BLOC
    CW = min(512, BT)
    assert T_BLK % 4 == 0

    nc = bass.Bass(trn_type="TRN2")

    gidx32 = nc.dram_tensor("gidx32", [128, NTOK // 128], mybir.dt.int32,
                            kind="ExternalInput")
    idxrows = nc.dram_tensor("idxrows", [7, NTOK], BF16, kind="ExternalInput")
    selpat = nc.dram_tensor("selpat", [7, 128], BF16, kind="ExternalInput")
    emb = nc.dram_tensor("emb", [VOC, EMB], BF16, kind="ExternalInput")
    wcembT = nc.dram_tensor("wcembT", [NT, EMB], BF16, kind="ExternalInput")
    wih = nc.dram_tensor("wih", [128, 16, 128], BF16, kind="ExternalInput")
    wihct = nc.dram_tensor("wihct", [NT, 8, 128], BF16, kind="ExternalInput")
    whh = nc.dram_tensor("whh", [128, 16, 128], BF16, kind="ExternalInput")
    bias8 = nc.dram_tensor("bias8", [128, 8], F32, kind="ExternalInput")
    ident = nc.dram_tensor("ident", [128, 128], BF16, kind="ExternalInput")
    woutT = nc.dram_tensor("woutT", [320, NUM_C], BF16, kind="ExternalInput")
    y = nc.dram_tensor("y", [NTOK, NUM_C], F32, kind="ExternalOutput")
    DBG = os.environ.get("DKT_DBG")
    if DBG:
        dbg_xe = nc.dram_tensor("dbg_xe", [2, 128, 2, BT], BF16,
                                kind="ExternalOutput")
        dbg_xg = nc.dram_tensor("dbg_xg", [2, 128, 8, BT], BF16,
                                kind="ExternalOutput")
        dbg_oh = nc.dram_tensor("dbg_oh", [2, NT, BT], BF16,
                                kind="ExternalOutput")
        dbg_xr = nc.dram_tensor("dbg_xr", [2, 128, BT // 128, EMB], BF16,
                                kind="ExternalOutput")

    with TileContext(nc) as tc:
        with tc.tile_pool(name="const", bufs=1) as cpool, \
             tc.tile_pool(name="blk", bufs=2) as bpool, \
             tc.tile_pool(name="sm", bufs=3) as spool, \
             tc.tile_pool(name="ps", bufs=2, space="PSUM") as pspool:
            selpat_sb = cpool.tile([7, 128], BF16)
            nc.sync.dma_start(out=selpat_sb, in_=selpat[:, :])
            ident_sb = cpool.tile([128, 128], BF16)
            nc.sync.dma_start(out=ident_sb, in_=ident[:, :])
            wcembT_sb = cpool.tile([NT, EMB], BF16)
            nc.sync.dma_start(out=wcembT_sb, in_=wcembT[:, :])
            wih_sb = cpool.tile([128, 16 * 128], BF16)
            nc.sync.dma_start(out=wih_sb, in_=wih.rearrange("k t m -> k (t m)"))
            wihct_sb = cpool.tile([NT, 8 * 128], BF16)
            nc.sync.dma_start(out=wihct_sb, in_=wihct.rearrange("k t m -> k (t m)"))
            whh_sb = cpool.tile([128, 16 * 128], BF16)
            nc.sync.dma_start(out=whh_sb, in_=whh.rearrange("k t m -> k (t m)"))
            bias_sb = cpool.tile([128, 8], F32)
            nc.sync.dma_start(out=bias_sb, in_=bias8[:, :])
            w0_sb = cpool.tile([128, NUM_C], BF16)
            nc.sync.dma_start(out=w0_sb, in_=woutT[0:128, :])
            w1_sb = cpool.tile([128, NUM_C], BF16)
            nc.sync.dma_start(out=w1_sb, in_=woutT[128:256, :])
            w2_sb = cpool.tile([NT, NUM_C], BF16)
            nc.sync.dma_start(out=w2_sb, in_=woutT[256:320, :])
            gidx_sb = cpool.tile([128, NTOK // 128], mybir.dt.int32)
            nc.sync.dma_start(out=gidx_sb, in_=gidx32[:, :])

            state = {"last_mm": None}

            def mm(out, lhsT, rhs, start, stop, is_transpose=None):
                m = nc.tensor.matmul(out, lhsT=lhsT, rhs=rhs, start=start,
                                     stop=stop, skip_group_check=True,
                                     is_transpose=is_transpose)
                if state["last_mm"] is not None:
                    add_dep_helper(m.ins, state["last_mm"].ins,
                                   reason="freeze PE order")
                state["last_mm"] = m
                return m

            NB128 = BT // 128  # 128-token sub-blocks per block

            # --- per-block phase-1 units ---
            def p1_rows_gather(l):
                """DMA idx rows + indirect-gather of embedding rows (token-
                major: xr[p, i, :] = emb[x[l*BT + 128i + p], :])."""
                t0 = l * BT
                rows = bpool.tile([7, BT], BF16, tag="rows", name="rows")
                nc.sync.dma_start(out=rows, in_=idxrows[:, t0:t0 + BT])
                xr = bpool.tile([128, NB128, EMB], BF16, tag="xr", name="xr")
                for i in range(NB128):
                    nc.gpsimd.indirect_dma_start(
                        out=xr[:, i, :], out_offset=None, in_=emb[:, :],
                        in_offset=bass.IndirectOffsetOnAxis(
                            ap=gidx_sb[:, l * NB128 + i:l * NB128 + i + 1],
                            axis=0))
                return rows, xr

            def p1_transpose_unit(xr, xe, q):
                """Transpose 4 raw 128-token sub-blocks (8 PE transposes) into
                xe[:, e, tokens] via one PSUM bank + one DVE copy."""
                pst = pspool.tile([128, 4, 2, 128], BF16, tag="xgps",
                                  name="pst")
                for s in range(4):
                    i = 4 * q + s
                    for e in range(2):
                        mm(pst[:, s, e, :], xr[:, i, 128 * e:128 * (e + 1)],
                           ident_sb, start=(s == 0 and e == 0),
                           stop=(s == 3 and e == 1), is_transpose=True)
                # dest: xe[:, e, 128*(4q+s) : +128] for each (s, e)
                dst = xe[:, :, 512 * q:512 * (q + 1)]
                dst = dst.rearrange("p e (s c) -> p s e c", s=4)
                nc.vector.tensor_copy(out=dst, in_=pst)

            def p1_oh_unit(rows, oh, which, c):
                """One CW-chunk of one-hot build for block tile `oh`."""
                sl = slice(CW * c, CW * (c + 1))
                ps = pspool.tile([NT, CW], F32, tag="cct", name="ohps")
                mm(ps, selpat_sb[:, 64 * which:64 * which + 64], rows[:, sl],
                   True, True)
                nc.vector.tensor_scalar(out=oh[:, sl], in0=ps, scalar1=0.0,
                                        scalar2=None, op0=ALU.is_equal)

            def p1_theta_unit(xe, oh_in, th, e, c):
                sl = slice(CW * c, CW * (c + 1))
                ps = pspool.tile([128, CW], F32, tag="cct", name="thps")
                mm(ps, wcembT_sb[:, 128 * e:128 * (e + 1)], oh_in[:, sl],
                   True, True)
                nc.vector.tensor_mul(out=th[:, e, sl], in0=xe[:, e, sl], in1=ps)

            def p1_xg_unit(th, oh_in, xg_t, j, c, use_act):
                sl = slice(CW * c, CW * (c + 1))
                ps = pspool.tile([128, CW], F32, tag="xgps", name="xgps")
                for k in range(3):
                    if k < 2:
                        lhsT = wih_sb[:, (k * 8 + j) * 128:(k * 8 + j + 1) * 128]
                        rhs = th[:, k, sl]
                    else:
                        lhsT = wihct_sb[:, j * 128:(j + 1) * 128]
                        rhs = oh_in[:, sl]
                    mm(ps, lhsT, rhs, k == 0, k == 2)
                if use_act:
                    nc.scalar.add(xg_t[:, j, sl], ps, bias_sb[:, j:j + 1])
                else:
                    nc.vector.tensor_scalar(
                        out=xg_t[:, j, sl], in0=ps,
                        scalar1=bias_sb[:, j:j + 1], scalar2=None,
                        op0=ALU.add)

            def p1_alloc_and_units(l):
                """Allocate block-l phase-1 tiles and return (tiles, units):
                units are thunks in producer-before-consumer order."""
                rows, xr = p1_rows_gather(l)
                xe = bpool.tile([128, 2, BT], BF16, tag="xe", name="xe")
                oh_in = bpool.tile([NT, BT], BF16, tag="ohin", name="oh_in")
                oh_out = bpool.tile([NT, BT], BF16, tag="ohout", name="oh_out")
                th = bpool.tile([128, 2, BT], BF16, tag="th", name="th")
                xg_t = bpool.tile([128, 8, BT], BF16, tag="xg", name="xg_t")
                units = []
                for q in range(NB128 // 4):
                    units.append(lambda q=q: p1_transpose_unit(xr, xe, q))
                for c in range(BT // CW):
                    units.append(lambda c=c: p1_oh_unit(rows, oh_in, 0, c))
                    units.append(lambda c=c: p1_oh_unit(rows, oh_out, 1, c))
                for e in range(2):
                    for c in range(BT // CW):
                        units.append(
                            lambda e=e, c=c: p1_theta_unit(xe, oh_in, th, e, c))
                for j in range(8):
                    for c in range(BT // CW):
                        units.append(
                            lambda j=j, c=c: p1_xg_unit(th, oh_in, xg_t, j, c,
                                                        use_act=False))
                if DBG and l < 2:
                    def dump():
                        nc.sync.dma_start(out=dbg_xe[l], in_=xe)
                        nc.sync.dma_start(out=dbg_xg[l], in_=xg_t)
                        nc.sync.dma_start(out=dbg_oh[l], in_=oh_in)
                        nc.sync.dma_start(out=dbg_xr[l], in_=xr)
                    units.append(dump)
                return (oh_in, oh_out, th, xg_t), units

            # --- phase-3 for one 128-token group (4 steps), split into
            # per-step units so the big output sigmoids never monopolize the
            # in-order ACT queue between two LSTM-chain ops ---
            def p3_units(hb, oh_out, l, u0):
                t0 = l * BT
                tsl = slice(32 * u0, 32 * (u0 + 4))
                tho = spool.tile([128, 2, 128], BF16, tag="tho", bufs=3,
                                 name="tho")
                ys = spool.tile([128, NUM_C], F32, tag="ys", bufs=2, name="ys")

                def u_tho():
                    for e in range(2):
                        psc = pspool.tile([128, 128], F32, tag="cct",
                                          name="cct3")
                        mm(psc, wcembT_sb[:, 128 * e:128 * (e + 1)],
                           oh_out[:, tsl], True, True)
                        nc.vector.tensor_mul(out=tho[:, e, :],
                                             in0=hb[:, e, tsl], in1=psc)

                def u_chunk(c, last):
                    sl = slice(512 * c, 512 * (c + 1))
                    psy = pspool.tile([128, 512], F32, tag="psy", name="psy")
                    for k in range(3):
                        lhsT = tho[:, k, :] if k < 2 else oh_out[:, tsl]
                        rhs = (w0_sb, w1_sb, w2_sb)[k][:, sl]
                        mm(psy, lhsT, rhs, k == 0, k == 2)
                    nc.scalar.activation(ys[:, sl], psy, AF.Sigmoid)
                    if last:
                        nc.sync.dma_start(
                            out=y[t0 + 32 * u0:t0 + 32 * (u0 + 4), :], in_=ys)

                return [u_tho] + [
                    (lambda c=c: u_chunk(c, c == NUM_C // 512 - 1))
                    for c in range(NUM_C // 512)]

            # --- prologue: phase-1 for block 0 ---
            cur, units0 = p1_alloc_and_units(0)
            for unit in units0:
                unit()
            hprev = None
            comb = spool.tile([128, 4, 32], F32, tag="comb", name="comb0")
            units = []
            p3q = []
            for l in range(NBLK):
                oh_in, oh_out, th, xg_t = cur
                hb = bpool.tile([128, 2, BT], BF16, tag="hb", name="hb")
                for u in range(T_BLK):
                    t = l * T_BLK + u
                    ps = pspool.tile([128, 8, 32], F32, tag="gates", name="gps")
                    mm(ps, ident_sb, xg_t[:, :, 32 * u:32 * (u + 1)],
                       True, t == 0)
                    if t > 0:
                        hbp, up = hprev
                        # k-major: all 8 matmuls on h-chunk 0 first, so they
                        # issue as soon as h0 is ready (h1 still computing)
                        for k in range(2):
                            for j in range(8):
                                mm(ps[:, j, :],
                                   whh_sb[:, (k * 8 + j) * 128:(k * 8 + j + 1) * 128],
                                   hbp[:, k, 32 * up:32 * (up + 1)],
                                   False, k == 1)
                    # comb tile holds [tanh(g_t) | c_{t-1}] so one fused DVE
                    # multiply produces [i*g | f*c]; comb for t+1 is allocated
                    # here and receives c_t from the adds below.
                    sg = spool.tile([128, 6, 32], BF16, tag="sg", name="sg")
                    nc.scalar.activation(sg, ps[:, 0:6, :], AF.Sigmoid)
                    comb_n = spool.tile([128, 4, 32], F32, tag="comb",
                                        name="comb_n")
                    nc.scalar.activation(comb[:, 0:2, :], ps[:, 6:8, :], AF.Tanh)
                    tct = spool.tile([128, 2, 32], BF16, tag="tct", name="tct")
                    if t == 0:
                        # c0 = i*g straight into next step's comb c-slot
                        nc.vector.tensor_mul(comb_n[:, 2:4, :], sg[:, 0:2, :],
                                             comb[:, 0:2, :])
                        nc.scalar.activation(tct, comb_n[:, 2:4, :], AF.Tanh)
                        nc.vector.tensor_mul(
                            hb[:, :, 32 * u:32 * (u + 1)], sg[:, 4:6, :], tct)
                    else:
                        prod = spool.tile([128, 4, 32], F32, tag="prod",
                                          name="prod")
                        nc.vector.tensor_mul(prod, sg[:, 0:4, :], comb)
                        nc.vector.tensor_add(comb_n[:, 2:4, :],
                                             prod[:, 0:2, :], prod[:, 2:4, :])
                        nc.scalar.activation(tct, comb_n[:, 2:4, :], AF.Tanh)
                        nc.vector.tensor_mul(
                            hb[:, :, 32 * u:32 * (u + 1)], sg[:, 4:6, :], tct)
                    hprev = (hb, u)
                    comb = comb_n

                    # ---- interleaved work for next block's phase 1 ----
                    if l + 1 < NBLK and not os.environ.get("DKT_SKIP_P1"):
                        if u == 0:
                            nxt_tiles, units = p1_alloc_and_units(l + 1)
                        else:
                            left = max(1, T_BLK - 1 - u)
                            npop = max(1, -(-len(units) // left)) \
                                if len(units) >= left else 1
                            for _ in range(npop):
                                if units:
                                    units.pop(0)()
                    if u % 4 == 3 and not os.environ.get("DKT_SKIP_P3"):
                        p3q.extend(p3_units(hb, oh_out, l, u - 3))
                    # drain ~1.25 phase-3 units per step
                    npop3 = 2 if len(p3q) > 5 else (1 if p3q else 0)
                    for _ in range(npop3):
                        if p3q:
                            p3q.pop(0)()
                while units:
                    units.pop(0)()
                if l + 1 < NBLK and not os.environ.get("DKT_SKIP_P1"):
                    cur = nxt_tiles
            while p3q:
                p3q.pop(0)()
    return nc


# ------------------------------------------------------------------
# host side
# ------------------------------------------------------------------

def _sel_patterns():
    pat = np.zeros((7, 128), np.float32)
    for which in range(2):
        o = 64 * which
        r = 3 * which
        pat[r + 0, o + 0:o + 16] = 1.0
        pat[r + 1, o + 16:o + 32] = 1.0
        pat[r + 2, o + 32:o + 64] = 1.0
        pat[6, o + 0:o + 16] = -np.arange(16)
        pat[6, o + 16:o + 32] = -np.arange(16)
        pat[6, o + 32:o + 64] = -np.arange(32)
    return pat.astype(BF)


def _tok(a):
    """[BLOC, S] -> [S*BLOC] in s-major token order."""
    return np.ascontiguousarray(a.T).reshape(-1)


def make_inputs(inputs, S=512, version=None):
    if version is None:
        version = KERNEL_VERSION
    """Build shared weight arrays + per-core in_maps from the full inputs."""
    NTOK = S * BLOC
    f32 = np.float32
    q = np.asarray(inputs["q"]).astype(np.int64)
    r = np.asarray(inputs["r"]).astype(np.int64)
    x = (q + NUM_C * r).astype(np.int32)
    rg = np.asarray(inputs["rgaps"]).astype(np.int32)
    sg = np.asarray(inputs["sgaps"]).astype(np.int32)
    pc = np.asarray(inputs["pcounts"]).astype(np.int32)
    srg = np.asarray(inputs["shft_rgaps"]).astype(np.int32)
    ssg = np.asarray(inputs["shft_sgaps"]).astype(np.int32)
    spc = np.asarray(inputs["shft_pcounts"]).astype(np.int32)
    E = np.asarray(inputs["E_inter"], f32)
    W_cemb = np.asarray(inputs["W_cemb"], f32)
    W_ih = np.asarray(inputs["W_ih"], f32)
    W_hh = np.asarray(inputs["W_hh"], f32)
    b = (np.asarray(inputs["b_ih"], f32) + np.asarray(inputs["b_hh"], f32))
    W_out = np.asarray(inputs["W_out"], f32)

    # gate reorder i,f,g,o -> i,f,o,g
    perm = np.r_[0:512, 768:1024, 512:768]
    Wih_p = W_ih[perm]
    Whh_p = W_hh[perm]
    bias_p = b[perm]

    def kmaj(A):  # [1024, 256] -> [128 k, 16 (kappa,j), 128 m]
        return np.ascontiguousarray(
            A.reshape(8, 128, 2, 128).transpose(3, 2, 0, 1)
        ).reshape(128, 16, 128).astype(BF)

    shared = {
        "selpat": _sel_patterns(),
        "emb": E.astype(BF),
        "wcembT": np.ascontiguousarray(W_cemb.T).astype(BF),
        "wih": kmaj(Wih_p[:, :256]),
        "wihct": np.ascontiguousarray(
            Wih_p[:, 256:320].reshape(8, 128, NT).transpose(2, 0, 1)).astype(BF),
        "whh": kmaj(Whh_p),
        "bias8": np.ascontiguousarray(bias_p.reshape(8, 128).T).astype(f32),
        "ident": np.eye(128).astype(BF),
        "woutT": np.ascontiguousarray(W_out.T).astype(BF),
    }

    in_maps = []
    ones = np.ones(NTOK, f32)
    for k in range(NCORES):
        bs = slice(BLOC * k, BLOC * (k + 1))
        tokv = _tok(x[bs])
        rows = np.stack([
            _tok(rg[bs]), _tok(sg[bs]), _tok(pc[bs]),
            _tok(srg[bs]), _tok(ssg[bs]), _tok(spc[bs]), ones]).astype(BF)
        m = dict(shared)
        if version == 2:
            # gidx32[p, g] = token index of token g*128 + p
            m["gidx32"] = np.ascontiguousarray(
                tokv.reshape(NTOK // 128, 128).T.astype(np.int32))
        else:
            g16 = tokv.astype(np.int16).reshape(NTOK // 16, 16).T
            m["gidx"] = np.ascontiguousarray(np.tile(g16, (8, 1)))
        m["idxrows"] = np.ascontiguousarray(rows)
        in_maps.append(m)
    return in_maps


_NC_CACHE = {}
KERNEL_VERSION = int(os.environ.get("DKT_KERNEL_V", "2")) if True else 2


def kernel(**inputs):
    S = np.asarray(inputs["q"]).shape[1]
    key = S
    if key not in _NC_CACHE:
        T_BLK = 64 if S % 64 == 0 else S
        build = build_nc_v2 if KERNEL_VERSION == 2 else build_nc
        nc_new = build(S=S, T_BLK=T_BLK)
        split_excess_waits(nc_new)
        _NC_CACHE[key] = nc_new
    nc = _NC_CACHE[key]
    in_maps = make_inputs(inputs, S=S)
    res = run_bass_kernel_spmd(nc, in_maps, core_ids=list(range(NCORES)))
    outs = []
    for k in range(NCORES):
        yk = res.results[k]["y"]  # [NTOK, NUM_C] token order s-major
        outs.append(yk.reshape(S, BLOC, NUM_C).transpose(1, 0, 2))
    return np.ascontiguousarray(np.concatenate(outs, axis=0))



# revision 28
# speedup vs baseline: 15711.9097x; 15711.9097x over previous
"""DKTForget Trainium2 kernel: embedding gather + one-hot c_integration +
LSTM scan + output projection, data-parallel over batch on 8 NeuronCores.

Self-contained: takes full (unsharded) inputs, shards batch across 8 cores,
runs one SPMD Bass kernel, gathers the full output.
"""
import os
import numpy as np
import ml_dtypes

import bass_rust
import concourse.bass as bass
import concourse.mybir as mybir
from concourse import library_config
from concourse.tile import TileContext, add_dep_helper
from concourse.bass_utils import run_bass_kernel_spmd

F32 = mybir.dt.float32
BF16 = mybir.dt.bfloat16
I16 = mybir.dt.int16
AF = mybir.ActivationFunctionType
ALU = mybir.AluOpType
BF = ml_dtypes.bfloat16

NUM_C = 2048
EMB = 256
NT = 64          # NUM_RGAP + NUM_SGAP + NUM_PCOUNT
H = 256
VOC = 2 * NUM_C  # 4096
B = 256
NCORES = 8
BLOC = B // NCORES  # 32 batch rows per core


def split_excess_waits(nc, maxw=1):
    """walrus CoreV3 allows a single sem-wait per instruction; split extras
    onto nop instructions inserted just before."""
    n_split = 0
    for f in nc.m.functions:
        for b in f.blocks:
            newlist = []
            for ins in b.instructions:
                si = ins.sync_info
                if si is not None and len(si.on_wait) > maxw:
                    w = list(si.on_wait)
                    k = 0
                    while len(w) - k > maxw:
                        nop = mybir.InstNoOp(name=f"{ins.name}_ws{k}", ins=[], outs=[])
                        nop.engine = ins.engine
                        nop.sync_info = bass_rust.SyncInfo(
                            on_wait=w[k:k + maxw], on_update=[])
                        newlist.append(nop)
                        k += maxw
                    ins.sync_info = bass_rust.SyncInfo(
                        on_wait=w[k:], on_update=list(si.on_update))
                    n_split += 1
                newlist.append(ins)
            b.instructions[:] = newlist
    return n_split


def build_nc(S=512, T_BLK=64):
    """Build the per-core Bass program (identical across cores)."""
    NTOK = S * BLOC
    NBLK = S // T_BLK
    BT = T_BLK * BLOC          # tokens per block
    GT = min(512, NTOK)        # phase-3 group tokens
    assert S % T_BLK == 0 and BT % 512 == 0 or BT % 128 == 0
    assert NTOK % GT == 0 and GT % 128 == 0

    nc = bass.Bass(trn_type="TRN2")

    # ---- I/O ----
    gidx = nc.dram_tensor("gidx", [128, NTOK // 16], I16, kind="ExternalInput")
    idxrows = nc.dram_tensor("idxrows", [7, NTOK], BF16, kind="ExternalInput")
    selpat = nc.dram_tensor("selpat", [7, 128], BF16, kind="ExternalInput")
    emb = nc.dram_tensor("emb", [VOC, EMB], BF16, kind="ExternalInput")
    wcembT = nc.dram_tensor("wcembT", [NT, EMB], BF16, kind="ExternalInput")
    wih = nc.dram_tensor("wih", [128, 16, 128], BF16, kind="ExternalInput")
    wihct = nc.dram_tensor("wihct", [NT, 8, 128], BF16, kind="ExternalInput")
    whh = nc.dram_tensor("whh", [128, 16, 128], BF16, kind="ExternalInput")
    bias8 = nc.dram_tensor("bias8", [128, 8], F32, kind="ExternalInput")
    ident = nc.dram_tensor("ident", [128, 128], BF16, kind="ExternalInput")
    woutT = nc.dram_tensor("woutT", [320, NUM_C], BF16, kind="ExternalInput")
    y = nc.dram_tensor("y", [NTOK, NUM_C], F32, kind="ExternalOutput")
    # internal DRAM spill for xg (precomputed input gates, transposed) and hT
    xgd = nc.dram_tensor("xgd", [128, 8, NTOK], BF16, kind="Internal")
    hTd = nc.dram_tensor("hTd", [128, 2, NTOK], BF16, kind="Internal")

    with TileContext(nc) as tc:
        with tc.tile_pool(name="const", bufs=1) as cpool, \
             tc.tile_pool(name="onehot", bufs=1) as ohpool:
            # ---- resident constants ----
            selpat_sb = cpool.tile([7, 128], BF16)
            nc.sync.dma_start(out=selpat_sb, in_=selpat[:, :])
            ident_sb = cpool.tile([128, 128], BF16)
            nc.sync.dma_start(out=ident_sb, in_=ident[:, :])
            wcembT_sb = cpool.tile([NT, EMB], BF16)
            nc.sync.dma_start(out=wcembT_sb, in_=wcembT[:, :])
            wih_sb = cpool.tile([128, 16 * 128], BF16)
            nc.sync.dma_start(out=wih_sb, in_=wih.rearrange("k t m -> k (t m)"))
            wihct_sb = cpool.tile([NT, 8 * 128], BF16)
            nc.sync.dma_start(out=wihct_sb, in_=wihct.rearrange("k t m -> k (t m)"))
            whh_sb = cpool.tile([128, 16 * 128], BF16)
            nc.sync.dma_start(out=whh_sb, in_=whh.rearrange("k t m -> k (t m)"))
            bias_sb = cpool.tile([128, 8], F32)
            nc.sync.dma_start(out=bias_sb, in_=bias8[:, :])
            w0_sb = cpool.tile([128, NUM_C], BF16)
            nc.sync.dma_start(out=w0_sb, in_=woutT[0:128, :])
            w1_sb = cpool.tile([128, NUM_C], BF16)
            nc.sync.dma_start(out=w1_sb, in_=woutT[128:256, :])
            w2_sb = cpool.tile([NT, NUM_C], BF16)
            nc.sync.dma_start(out=w2_sb, in_=woutT[256:320, :])
            gidx_sb = cpool.tile([128, NTOK // 16], I16)
            nc.sync.dma_start(out=gidx_sb, in_=gidx[:, :])

            # persistent one-hot^T buffers [NT, NTOK]
            oh_in = ohpool.tile([NT, NTOK], BF16)
            oh_out = ohpool.tile([NT, NTOK], BF16)

            # ---- P0: build one-hots for all tokens ----
            with tc.tile_pool(name="p0", bufs=2) as p0pool, \
                 tc.tile_pool(name="p0ps", bufs=2, space="PSUM") as p0ps:
                rows_sb = cpool.tile([7, NTOK], BF16)
                nc.sync.dma_start(out=rows_sb, in_=idxrows[:, :])
                CH = min(2048, NTOK)
                CW0 = min(512, CH)
                for ci in range(NTOK // CH):
                    sl = slice(ci * CH, (ci + 1) * CH)
                    for which, oh in ((0, oh_in), (1, oh_out)):
                        ps = p0ps.tile([NT, CH], F32, tag="idxps")
                        for c in range(CH // CW0):
                            nc.tensor.matmul(
                                ps[:, CW0 * c:CW0 * (c + 1)],
                                lhsT=selpat_sb[:, 64 * which:64 * which + 64],
                                rhs=rows_sb[:, ci * CH + CW0 * c: ci * CH + CW0 * (c + 1)],
                                start=True, stop=True)
                        nc.vector.tensor_scalar(
                            out=oh[:, sl], in0=ps, scalar1=0.0, scalar2=None,
                            op0=ALU.is_equal)

            # ---- P1: gather + theta_in + xg ----
            with tc.tile_pool(name="p1", bufs=2) as p1pool, \
                 tc.tile_pool(name="p1ps", bufs=2, space="PSUM") as p1ps:
                nc.gpsimd.load_library(library_config.mlp)
                CW = min(512, BT)
                for l in range(NBLK):
                    t0 = l * BT
                    xe = p1pool.tile([128, 2, BT], BF16, tag="xe")
                    nc.gpsimd.dma_gather(
                        xe, emb[:, :], gidx_sb[:, t0 // 16:(t0 + BT) // 16],
                        num_idxs=BT, num_idxs_reg=BT, elem_size=EMB,
                        transpose=True)
                    th = p1pool.tile([128, 2, BT], BF16, tag="th")
                    for e in range(2):
                        for c in range(BT // CW):
                            sl = slice(CW * c, CW * (c + 1))
                            gsl = slice(t0 + CW * c, t0 + CW * (c + 1))
                            psc = p1ps.tile([128, CW], F32, tag="cct")
                            nc.tensor.matmul(
                                psc, lhsT=wcembT_sb[:, 128 * e:128 * (e + 1)],
                                rhs=oh_in[:, gsl], start=True, stop=True)
                            nc.vector.tensor_mul(
                                out=th[:, e, sl], in0=xe[:, e, sl], in1=psc)
                    for j in range(8):
                        xs = p1pool.tile([128, BT], BF16, tag="xs")
                        for c in range(BT // CW):
                            sl = slice(CW * c, CW * (c + 1))
                            gsl = slice(t0 + CW * c, t0 + CW * (c + 1))
                            psx = p1ps.tile([128, CW], F32, tag="xg")
                            for k in range(3):
                                if k < 2:
                                    lhsT = wih_sb[:, (k * 8 + j) * 128:(k * 8 + j + 1) * 128]
                                    rhs = th[:, k, sl]
                                else:
                                    lhsT = wihct_sb[:, j * 128:(j + 1) * 128]
                                    rhs = oh_in[:, gsl]
                                nc.tensor.matmul(psx, lhsT=lhsT, rhs=rhs,
                                                 start=(k == 0), stop=(k == 2))
                            if (j + c) % 2 == 0:
                                nc.scalar.add(xs[:, sl], psx, bias_sb[:, j:j + 1])
                            else:
                                nc.vector.tensor_scalar(
                                    out=xs[:, sl], in0=psx,
                                    scalar1=bias_sb[:, j:j + 1], scalar2=None,
                                    op0=ALU.add)
                        nc.sync.dma_start(out=xgd[:, j, t0:t0 + BT], in_=xs)

            # ---- P2: LSTM scan ----
            with tc.tile_pool(name="p2", bufs=2) as p2pool, \
                 tc.tile_pool(name="p2s", bufs=3) as p2s, \
                 tc.tile_pool(name="p2ps", bufs=2, space="PSUM") as p2ps:
                hprev = None  # (tile, u) of previous step
                cprev = None
                last_mm = None  # previous step's final matmul (pin group order)
                for l in range(NBLK):
                    t0 = l * BT
                    xg_sb = p2pool.tile([128, 8, BT], BF16, tag="xgl")
                    nc.sync.dma_start(out=xg_sb, in_=xgd[:, :, t0:t0 + BT])
                    hb = p2pool.tile([128, 2, BT], BF16, tag="hb")
                    for u in range(T_BLK):
                        t = l * T_BLK + u
                        ps = p2ps.tile([128, 8, 32], F32, tag="gates")
                        mm = nc.tensor.matmul(
                            ps, lhsT=ident_sb, rhs=xg_sb[:, :, 32 * u:32 * (u + 1)],
                            start=True, stop=(t == 0), skip_group_check=True)
                        if last_mm is not None:
                            add_dep_helper(mm.ins, last_mm.ins,
                                           reason="keep psum groups contiguous")
                        last_mm = mm
                        if t > 0:
                            hbp, up = hprev
                            for j in range(8):
                                for k in range(2):
                                    last_mm = nc.tensor.matmul(
                                        ps[:, j, :],
                                        lhsT=whh_sb[:, (k * 8 + j) * 128:(k * 8 + j + 1) * 128],
                                        rhs=hbp[:, k, 32 * up:32 * (up + 1)],
                                        start=False, stop=(k == 1),
                                        skip_group_check=True)
                        sg = p2s.tile([128, 6, 32], BF16, tag="sg")
                        nc.scalar.activation(sg, ps[:, 0:6, :], AF.Sigmoid)
                        gg = p2s.tile([128, 2, 32], BF16, tag="gg")
                        nc.scalar.activation(gg, ps[:, 6:8, :], AF.Tanh)
                        cn = p2s.tile([128, 2, 32], F32, tag="cn")
                        if t == 0:
                            nc.vector.tensor_mul(cn, sg[:, 0:2, :], gg)
                        else:
                            m = p2s.tile([128, 2, 32], F32, tag="m")
                            nc.vector.tensor_mul(m, sg[:, 0:2, :], gg)
                            cf = p2s.tile([128, 2, 32], F32, tag="cf")
                            nc.vector.tensor_mul(cf, cprev, sg[:, 2:4, :])
                            nc.vector.tensor_add(cn, cf, m)
                        tct = p2s.tile([128, 2, 32], BF16, tag="tct")
                        nc.scalar.activation(tct, cn, AF.Tanh)
                        nc.vector.tensor_mul(
                            hb[:, :, 32 * u:32 * (u + 1)], sg[:, 4:6, :], tct)
                        hprev = (hb, u)
                        cprev = cn
                    nc.sync.dma_start(out=hTd[:, :, t0:t0 + BT], in_=hb)

            # ---- P3: theta_out + output matmul + sigmoid ----
            with tc.tile_pool(name="p3", bufs=2) as p3pool, \
                 tc.tile_pool(name="p3y", bufs=2) as p3y, \
                 tc.tile_pool(name="p3ps", bufs=2, space="PSUM") as p3ps, \
                 tc.tile_pool(name="p3yps", bufs=4, space="PSUM") as p3yps:
                GT_ = min(GT, BT)
                for l in range(NBLK):
                    t0 = l * BT
                    h3 = p3pool.tile([128, 2, BT], BF16, tag="h3")
                    nc.sync.dma_start(out=h3, in_=hTd[:, :, t0:t0 + BT])
                    for gi in range(BT // GT_):
                        g0 = t0 + gi * GT_
                        tho = p3pool.tile([128, 2, GT_], BF16, tag="tho")
                        for e in range(2):
                            psc = p3ps.tile([128, GT_], F32, tag="cct3")
                            w = min(512, GT_)
                            for c in range(GT_ // w):
                                nc.tensor.matmul(
                                    psc[:, w * c:w * (c + 1)],
                                    lhsT=wcembT_sb[:, 128 * e:128 * (e + 1)],
                                    rhs=oh_out[:, g0 + w * c:g0 + w * (c + 1)],
                                    start=True, stop=True)
                            nc.vector.tensor_mul(
                                out=tho[:, e, :],
                                in0=h3[:, e, gi * GT_:(gi + 1) * GT_], in1=psc)
                        for sub in range(GT_ // 128):
                            s0 = g0 + 128 * sub
                            ys = p3y.tile([128, NUM_C], F32, tag="ys")
                            for c in range(NUM_C // 512):
                                sl = slice(512 * c, 512 * (c + 1))
                                psy = p3yps.tile([128, 512], F32, tag="psy")
                                for k in range(3):
                                    if k < 2:
                                        lhsT = tho[:, k, 128 * sub:128 * (sub + 1)]
                                    else:
                                        lhsT = oh_out[:, s0:s0 + 128]
                                    rhs = (w0_sb, w1_sb, w2_sb)[k][:, sl]
                                    nc.tensor.matmul(psy, lhsT=lhsT, rhs=rhs,
                                                     start=(k == 0), stop=(k == 2))
                                nc.scalar.activation(ys[:, sl], psy, AF.Sigmoid)
                            nc.sync.dma_start(out=y[s0:s0 + 128, :], in_=ys)

    return nc


def build_nc_v2(S=512, T_BLK=64):
    """Interleaved single-pass build: phase-1 (gather/theta/xg for block l+1)
    and phase-3 (output projection for finished steps) are woven between the
    LSTM steps of block l, so PE/ACT stall windows of the serial recurrence do
    the bulk work. No DRAM spills: xg and h stay in SBUF."""
    NTOK = S * BLOC
    NBLK = S // T_BLK
    BT = T_BLK * BLOC
    CW = min(512, BT)
    assert T_BLK % 4 == 0

    nc = bass.Bass(trn_type="TRN2")

    gidx32 = nc.dram_tensor("gidx32", [128, NTOK // 128], mybir.dt.int32,
                            kind="ExternalInput")
    idxrows = nc.dram_tensor("idxrows", [7, NTOK], BF16, kind="ExternalInput")
    selpat = nc.dram_tensor("selpat", [7, 128], BF16, kind="ExternalInput")
    emb = nc.dram_tensor("emb", [VOC, EMB], BF16, kind="ExternalInput")
    wcembT = nc.dram_tensor("wcembT", [NT, EMB], BF16, kind="ExternalInput")
    wih = nc.dram_tensor("wih", [128, 16, 128], BF16, kind="ExternalInput")
    wihct = nc.dram_tensor("wihct", [NT, 8, 128], BF16, kind="ExternalInput")
    whh = nc.dram_tensor("whh", [128, 16, 128], BF16, kind="ExternalInput")
    bias8 = nc.dram_tensor("bias8", [128, 8], F32, kind="ExternalInput")
    ident = nc.dram_tensor("ident", [128, 128], BF16, kind="ExternalInput")
    woutT = nc.dram_tensor("woutT", [320, NUM_C], BF16, kind="ExternalInput")
    y = nc.dram_tensor("y", [NTOK, NUM_C], F32, kind="ExternalOutput")
    DBG = os.environ.get("DKT_DBG")
    if DBG:
        dbg_xe = nc.dram_tensor("dbg_xe", [2, 128, 2, BT], BF16,
                                kind="ExternalOutput")
        dbg_xg = nc.dram_tensor("dbg_xg", [2, 128, 8, BT], BF16,
                                kind="ExternalOutput")
        dbg_oh = nc.dram_tensor("dbg_oh", [2, NT, BT], BF16,
                                kind="ExternalOutput")
        dbg_xr = nc.dram_tensor("dbg_xr", [2, 128, BT // 128, EMB], BF16,
                                kind="ExternalOutput")

    with TileContext(nc) as tc:
        with tc.tile_pool(name="const", bufs=1) as cpool, \
             tc.tile_pool(name="blk", bufs=2) as bpool, \
             tc.tile_pool(name="sm", bufs=3) as spool, \
             tc.tile_pool(name="ps", bufs=2, space="PSUM") as pspool:
            selpat_sb = cpool.tile([7, 128], BF16)
            nc.sync.dma_start(out=selpat_sb, in_=selpat[:, :])
            ident_sb = cpool.tile([128, 128], BF16)
            nc.sync.dma_start(out=ident_sb, in_=ident[:, :])
            wcembT_sb = cpool.tile([NT, EMB], BF16)
            nc.sync.dma_start(out=wcembT_sb, in_=wcembT[:, :])
            wih_sb = cpool.tile([128, 16 * 128], BF16)
            nc.sync.dma_start(out=wih_sb, in_=wih.rearrange("k t m -> k (t m)"))
            wihct_sb = cpool.tile([NT, 8 * 128], BF16)
            nc.sync.dma_start(out=wihct_sb, in_=wihct.rearrange("k t m -> k (t m)"))
            whh_sb = cpool.tile([128, 16 * 128], BF16)
            nc.sync.dma_start(out=whh_sb, in_=whh.rearrange("k t m -> k (t m)"))
            bias_sb = cpool.tile([128, 8], F32)
            nc.sync.dma_start(out=bias_sb, in_=bias8[:, :])
            w0_sb = cpool.tile([128, NUM_C], BF16)
            nc.sync.dma_start(out=w0_sb, in_=woutT[0:128, :])
            w1_sb = cpool.tile([128, NUM_C], BF16)
            nc.sync.dma_start(out=w1_sb, in_=woutT[128:256, :])
            w2_sb = cpool.tile([NT, NUM_C], BF16)
            nc.sync.dma_start(out=w2_sb, in_=woutT[256:320, :])
            gidx_sb = cpool.tile([128, NTOK // 128], mybir.dt.int32)
            nc.sync.dma_start(out=gidx_sb, in_=gidx32[:, :])

            state = {"last_mm": None}

            def mm(out, lhsT, rhs, start, stop, is_transpose=None):
                m = nc.tensor.matmul(out, lhsT=lhsT, rhs=rhs, start=start,
                                     stop=stop, skip_group_check=True,
                                     is_transpose=is_transpose)
                if state["last_mm"] is not None:
                    add_dep_helper(m.ins, state["last_mm"].ins,
                                   reason="freeze PE order")
                state["last_mm"] = m
                return m

            NB128 = BT // 128  # 128-token sub-blocks per block

            # --- per-block phase-1 units ---
            def p1_rows_gather(l):
                """DMA idx rows + indirect-gather of embedding rows (token-
                major: xr[p, i, :] = emb[x[l*BT + 128i + p], :])."""
                t0 = l * BT
                rows = bpool.tile([7, BT], BF16, tag="rows", name="rows")
                nc.sync.dma_start(out=rows, in_=idxrows[:, t0:t0 + BT])
                xr = bpool.tile([128, NB128, EMB], BF16, tag="xr", name="xr")
                for i in range(NB128):
                    nc.gpsimd.indirect_dma_start(
                        out=xr[:, i, :], out_offset=None, in_=emb[:, :],
                        in_offset=bass.IndirectOffsetOnAxis(
                            ap=gidx_sb[:, l * NB128 + i:l * NB128 + i + 1],
                            axis=0))
                return rows, xr

            def p1_transpose_unit(xr, xe, q):
                """Transpose 4 raw 128-token sub-blocks (8 PE transposes) into
                xe[:, e, tokens] via one PSUM bank + one DVE copy."""
                pst = pspool.tile([128, 4, 2, 128], BF16, tag="xgps",
                                  name="pst")
                for s in range(4):
                    i = 4 * q + s
                    for e in range(2):
                        mm(pst[:, s, e, :], xr[:, i, 128 * e:128 * (e + 1)],
                           ident_sb, start=(s == 0 and e == 0),
                           stop=(s == 3 and e == 1), is_transpose=True)
                # dest: xe[:, e, 128*(4q+s) : +128] for each (s, e)
                dst = xe[:, :, 512 * q:512 * (q + 1)]
                dst = dst.rearrange("p e (s c) -> p s e c", s=4)
                nc.vector.tensor_copy(out=dst, in_=pst)

            def p1_oh_unit(rows, oh, which, c):
                """One CW-chunk of one-hot build for block tile `oh`."""
                sl = slice(CW * c, CW * (c + 1))
                ps = pspool.tile([NT, CW], F32, tag="cct", name="ohps")
                mm(ps, selpat_sb[:, 64 * which:64 * which + 64], rows[:, sl],
                   True, True)
                nc.vector.tensor_scalar(out=oh[:, sl], in0=ps, scalar1=0.0,
                                        scalar2=None, op0=ALU.is_equal)

            def p1_theta_unit(xe, oh_in, th, e, c):
                sl = slice(CW * c, CW * (c + 1))
                ps = pspool.tile([128, CW], F32, tag="cct", name="thps")
                mm(ps, wcembT_sb[:, 128 * e:128 * (e + 1)], oh_in[:, sl],
                   True, True)
                nc.vector.tensor_mul(out=th[:, e, sl], in0=xe[:, e, sl], in1=ps)

            def p1_xg_unit(th, oh_in, xg_t, j, c, use_act):
                sl = slice(CW * c, CW * (c + 1))
                ps = pspool.tile([128, CW], F32, tag="xgps", name="xgps")
                for k in range(3):
                    if k < 2:
                        lhsT = wih_sb[:, (k * 8 + j) * 128:(k * 8 + j + 1) * 128]
                        rhs = th[:, k, sl]
                    else:
                        lhsT = wihct_sb[:, j * 128:(j + 1) * 128]
                        rhs = oh_in[:, sl]
                    mm(ps, lhsT, rhs, k == 0, k == 2)
                if use_act:
                    nc.scalar.add(xg_t[:, j, sl], ps, bias_sb[:, j:j + 1])
                else:
                    nc.vector.tensor_scalar(
                        out=xg_t[:, j, sl], in0=ps,
                        scalar1=bias_sb[:, j:j + 1], scalar2=None,
                        op0=ALU.add)

            def p1_alloc_and_units(l):
                """Allocate block-l phase-1 tiles and return (tiles, units):
                units are thunks in producer-before-consumer order."""
                rows, xr = p1_rows_gather(l)
                xe = bpool.tile([128, 2, BT], BF16, tag="xe", name="xe")
                oh_in = bpool.tile([NT, BT], BF16, tag="ohin", name="oh_in")
                oh_out = bpool.tile([NT, BT], BF16, tag="ohout", name="oh_out")
                th = bpool.tile([128, 2, BT], BF16, tag="th", name="th")
                xg_t = bpool.tile([128, 8, BT], BF16, tag="xg", name="xg_t")
                units = []
                for q in range(NB128 // 4):
                    units.append(lambda q=q: p1_transpose_unit(xr, xe, q))
                for c in range(BT // CW):
                    units.append(lambda c=c: p1_oh_unit(rows, oh_in, 0, c))
                    units.append(lambda c=c: p1_oh_unit(rows, oh_out, 1, c))
                for e in range(2):
                    for c in range(BT // CW):
                        units.append(
                            lambda e=e, c=c: p1_theta_unit(xe, oh_in, th, e, c))
                for j in range(8):
                    for c in range(BT // CW):
                        units.append(
                            lambda j=j, c=c: p1_xg_unit(th, oh_in, xg_t, j, c,
                                                        use_act=False))
                if DBG and l < 2:
                    def dump():
                        nc.sync.dma_start(out=dbg_xe[l], in_=xe)
                        nc.sync.dma_start(out=dbg_xg[l], in_=xg_t)
                        nc.sync.dma_start(out=dbg_oh[l], in_=oh_in)
                        nc.sync.dma_start(out=dbg_xr[l], in_=xr)
                    units.append(dump)
                return (oh_in, oh_out, th, xg_t), units

            # --- phase-3 for one 128-token group (4 steps), split into
            # per-step units so the big output sigmoids never monopolize the
            # in-order ACT queue between two LSTM-chain ops ---
            def p3_units(hb, oh_out, l, u0):
                t0 = l * BT
                tsl = slice(32 * u0, 32 * (u0 + 4))
                tho = spool.tile([128, 2, 128], BF16, tag="tho", bufs=3,
                                 name="tho")
                ys = spool.tile([128, NUM_C], F32, tag="ys", bufs=2, name="ys")

                def u_tho():
                    for e in range(2):
                        psc = pspool.tile([128, 128], F32, tag="cct",
                                          name="cct3")
                        mm(psc, wcembT_sb[:, 128 * e:128 * (e + 1)],
                           oh_out[:, tsl], True, True)
                        nc.vector.tensor_mul(out=tho[:, e, :],
                                             in0=hb[:, e, tsl], in1=psc)

                def u_chunk(c, last):
                    sl = slice(512 * c, 512 * (c + 1))
                    psy = pspool.tile([128, 512], F32, tag="psy", name="psy")
                    for k in range(3):
                        lhsT = tho[:, k, :] if k < 2 else oh_out[:, tsl]
                        rhs = (w0_sb, w1_sb, w2_sb)[k][:, sl]
                        mm(psy, lhsT, rhs, k == 0, k == 2)
                    nc.scalar.activation(ys[:, sl], psy, AF.Sigmoid)
                    if last:
                        nc.sync.dma_start(
                            out=y[t0 + 32 * u0:t0 + 32 * (u0 + 4), :], in_=ys)

                return [u_tho] + [
                    (lambda c=c: u_chunk(c, c == NUM_C // 512 - 1))
                    for c in range(NUM_C // 512)]

            # --- prologue: phase-1 for block 0 ---
            cur, units0 = p1_alloc_and_units(0)
            for unit in units0:
                unit()
            hprev = None
            comb = spool.tile([128, 4, 32], F32, tag="comb", name="comb0")
            units = []
            p3q = []
            for l in range(NBLK):
                oh_in, oh_out, th, xg_t = cur
                hb = bpool.tile([128, 2, BT], BF16, tag="hb", name="hb")
                for u in range(T_BLK):
                    t = l * T_BLK + u
                    ps = pspool.tile([128, 8, 32], F32, tag="gates", name="gps")
                    mm(ps, ident_sb, xg_t[:, :, 32 * u:32 * (u + 1)],
                       True, t == 0)
                    if t > 0:
                        hbp, up = hprev
                        # k-major: all 8 matmuls on h-chunk 0 first, so they
                        # issue as soon as h0 is ready (h1 still computing)
                        for k in range(2):
                            for j in range(8):
                                mm(ps[:, j, :],
                                   whh_sb[:, (k * 8 + j) * 128:(k * 8 + j + 1) * 128],
                                   hbp[:, k, 32 * up:32 * (up + 1)],
                                   False, k == 1)
                    # comb tile holds [tanh(g_t) | c_{t-1}] so one fused DVE
                    # multiply produces [i*g | f*c]; comb for t+1 is allocated
                    # here and receives c_t from the adds below.
                    sg = spool.tile([128, 6, 32], BF16, tag="sg", name="sg")
                    nc.scalar.activation(sg, ps[:, 0:6, :], AF.Sigmoid)
                    comb_n = spool.tile([128, 4, 32], F32, tag="comb",
                                        name="comb_n")
                    nc.scalar.activation(comb[:, 0:2, :], ps[:, 6:8, :], AF.Tanh)
                    tct = spool.tile([128, 2, 32], BF16, tag="tct", name="tct")
                    if t == 0:
                        # c0 = i*g straight into next step's comb c-slot
                        nc.vector.tensor_mul(comb_n[:, 2:4, :], sg[:, 0:2, :],
                                             comb[:, 0:2, :])
                        nc.scalar.activation(tct, comb_n[:, 2:4, :], AF.Tanh)
                        nc.vector.tensor_mul(
                            hb[:, :, 32 * u:32 * (u + 1)], sg[:, 4:6, :], tct)
                    else:
                        prod = spool.tile([128, 4, 32], F32, tag="prod",
                                          name="prod")
                        nc.vector.tensor_mul(prod, sg[:, 0:4, :], comb)
                        nc.vector.tensor_add(comb_n[:, 2:4, :],
                                             prod[:, 0:2, :], prod[:, 2:4, :])
                        nc.scalar.activation(tct, comb_n[:, 2:4, :], AF.Tanh)
                        nc.vector.tensor_mul(
                            hb[:, :, 32 * u:32 * (u + 1)], sg[:, 4:6, :], tct)
                    hprev = (hb, u)
                    comb = comb_n

                    # ---- interleaved work for next block's phase 1 ----
                    if l + 1 < NBLK and not os.environ.get("DKT_SKIP_P1"):
                        if u == 0:
                            nxt_tiles, units = p1_alloc_and_units(l + 1)
                        else:
                            left = max(1, T_BLK - 1 - u)
                            npop = max(1, -(-len(units) // left)) \
                                if len(units) >= left else 1
                            for _ in range(npop):
                                if units:
                                    units.pop(0)()
                    if u % 4 == 3 and not os.environ.get("DKT_SKIP_P3"):
                        p3q.extend(p3_units(hb, oh_out, l, u - 3))
                    # drain ~1.25 phase-3 units per step
                    npop3 = 2 if len(p3q) > 5 else (1 if p3q else 0)
                    for _ in range(npop3):
                        if p3q:
                            p3q.pop(0)()
                while units:
                    units.pop(0)()
                if l + 1 < NBLK and not os.environ.get("DKT_SKIP_P1"):
                    cur = nxt_tiles
            while p3q:
                p3q.pop(0)()
    return nc


def build_nc_v3(S=512, T_BLK=64):
    """v3: single-pass interleave like v2, restructured so the LSTM chain is
    never queue-blocked: bulk (phase-1/phase-3) work is split into PE parts
    and non-PE tails, with tails drained one step after their producers, and
    all bulk emission happens strictly after the chain ops of each step.
    Gate math: tanh(g) = 2*sigmoid(2g) - 1 with the 2x folded into the g-gate
    weights, so one sigmoid covers all 8 gate groups; i,f,o,g sigmoids plus
    the carry c live in one [128,10,32] tile so the cell update is 2 DVE ops.
    LSTM bias is folded into the rgap rows of wihct (one-hot rows sum to 1).
    """
    NTOK = S * BLOC
    NBLK = S // T_BLK
    BT = T_BLK * BLOC
    CW = min(512, BT)
    assert T_BLK % 16 == 0 and BT % 512 == 0

    nc = bass.Bass(trn_type="TRN2")

    gidx32 = nc.dram_tensor("gidx32", [128, NTOK // 128], mybir.dt.int32,
                            kind="ExternalInput")
    idxrows = nc.dram_tensor("idxrows", [7, NTOK], BF16, kind="ExternalInput")
    selpat = nc.dram_tensor("selpat", [7, 128], BF16, kind="ExternalInput")
    emb = nc.dram_tensor("emb", [VOC, EMB], BF16, kind="ExternalInput")
    wcembT = nc.dram_tensor("wcembT", [NT, EMB], BF16, kind="ExternalInput")
    wih = nc.dram_tensor("wih", [128, 16, 128], BF16, kind="ExternalInput")
    wihct = nc.dram_tensor("wihct", [NT, 8, 128], BF16, kind="ExternalInput")
    whh = nc.dram_tensor("whh", [128, 16, 128], BF16, kind="ExternalInput")
    ident = nc.dram_tensor("ident", [128, 128], BF16, kind="ExternalInput")
    woutT = nc.dram_tensor("woutT", [320, NUM_C], BF16, kind="ExternalInput")
    y = nc.dram_tensor("y", [NTOK, NUM_C], F32, kind="ExternalOutput")
    DBG = os.environ.get("DKT_DBG")
    if DBG:
        dbg_xg = nc.dram_tensor("dbg_xg", [128, 8, BT], BF16,
                                kind="ExternalOutput")
        dbg_hb = nc.dram_tensor("dbg_hb", [128, 2, BT], BF16,
                                kind="ExternalOutput")
        dbg_oh = nc.dram_tensor("dbg_oh", [2, NT, BT], BF16,
                                kind="ExternalOutput")
        dbg_sg = nc.dram_tensor("dbg_sg", [4, 128, 10, 32], F32,
                                kind="ExternalOutput")
        dbg_tho = nc.dram_tensor("dbg_tho", [128, 2, 512], BF16,
                                 kind="ExternalOutput")

    with TileContext(nc) as tc:
        with tc.tile_pool(name="const", bufs=1) as cpool, \
             tc.tile_pool(name="blk", bufs=2) as bpool, \
             tc.tile_pool(name="sm", bufs=3) as spool, \
             tc.tile_pool(name="ps", bufs=2, space="PSUM") as pspool:
            selpat_sb = cpool.tile([7, 128], BF16)
            nc.sync.dma_start(out=selpat_sb, in_=selpat[:, :])
            ident_sb = cpool.tile([128, 128], BF16)
            nc.sync.dma_start(out=ident_sb, in_=ident[:, :])
            wcembT_sb = cpool.tile([NT, EMB], BF16)
            nc.sync.dma_start(out=wcembT_sb, in_=wcembT[:, :])
            wih_sb = cpool.tile([128, 16 * 128], BF16)
            nc.sync.dma_start(out=wih_sb, in_=wih.rearrange("k t m -> k (t m)"))
            wihct_sb = cpool.tile([NT, 8 * 128], BF16)
            nc.sync.dma_start(out=wihct_sb, in_=wihct.rearrange("k t m -> k (t m)"))
            whh_sb = cpool.tile([128, 16 * 128], BF16)
            nc.sync.dma_start(out=whh_sb, in_=whh.rearrange("k t m -> k (t m)"))
            w0_sb = cpool.tile([128, NUM_C], BF16)
            nc.sync.dma_start(out=w0_sb, in_=woutT[0:128, :])
            w1_sb = cpool.tile([128, NUM_C], BF16)
            nc.sync.dma_start(out=w1_sb, in_=woutT[128:256, :])
            w2_sb = cpool.tile([NT, NUM_C], BF16)
            nc.sync.dma_start(out=w2_sb, in_=woutT[256:320, :])
            gidx_sb = cpool.tile([128, NTOK // 128], mybir.dt.int32)
            nc.sync.dma_start(out=gidx_sb, in_=gidx32[:, :])

            state = {"last_mm": None}

            def mm(out, lhsT, rhs, start, stop, is_transpose=None):
                m = nc.tensor.matmul(out, lhsT=lhsT, rhs=rhs, start=start,
                                     stop=stop, skip_group_check=True,
                                     is_transpose=is_transpose)
                if state["last_mm"] is not None:
                    add_dep_helper(m.ins, state["last_mm"].ins,
                                   reason="freeze PE order")
                state["last_mm"] = m
                return m

            NB128 = BT // 128

            # ---- phase-1 units: (pe_fn, tail_fn) pairs ----
            def p1_alloc_units(l):
                t0 = l * BT
                rows = bpool.tile([7, BT], BF16, tag="rows", name="rows")
                nc.sync.dma_start(out=rows, in_=idxrows[:, t0:t0 + BT])
                xr = bpool.tile([128, NB128, EMB], BF16, tag="xr", name="xr")
                for i in range(NB128):
                    nc.gpsimd.indirect_dma_start(
                        out=xr[:, i, :], out_offset=None, in_=emb[:, :],
                        in_offset=bass.IndirectOffsetOnAxis(
                            ap=gidx_sb[:, l * NB128 + i:l * NB128 + i + 1],
                            axis=0))
                xe = bpool.tile([128, 2, BT], BF16, tag="xe", name="xe")
                oh_in = bpool.tile([NT, BT], BF16, tag="ohin", name="oh_in")
                # bufs=3: oh_out(l) is read by block-l phase-3 whose last
                # group drains into block l+1, while oh_out(l+2) is written
                # early in block l+1 -> with bufs=2 the slot-wait lands
                # behind newer matmuls in the frozen PE order (deadlock).
                oh_out = bpool.tile([NT, BT], BF16, tag="ohout", name="oh_out",
                                    bufs=3)
                th = bpool.tile([128, 2, BT], BF16, tag="th", name="th")
                xg_t = bpool.tile([128, 8, BT], BF16, tag="xg", name="xg_t")

                def oh_pe(which, c, psref):
                    sl = slice(CW * c, CW * (c + 1))
                    ps = pspool.tile([NT, CW], F32, tag="cct", name="ohps")
                    psref.append(ps)
                    mm(ps, selpat_sb[:, 64 * which:64 * which + 64],
                       rows[:, sl], True, True)

                def oh_tail(which, c, psref):
                    oh = oh_in if which == 0 else oh_out
                    sl = slice(CW * c, CW * (c + 1))
                    return nc.vector.tensor_scalar(out=oh[:, sl], in0=psref[0],
                                                   scalar1=0.0, scalar2=None,
                                                   op0=ALU.is_equal)

                def trans_pe(q, psref):
                    ps = pspool.tile([128, 4, 2, 128], BF16, tag="xgps",
                                     name="pst")
                    psref.append(ps)
                    for s in range(4):
                        i = 4 * q + s
                        for e in range(2):
                            mm(ps[:, s, e, :],
                               xr[:, i, 128 * e:128 * (e + 1)], ident_sb,
                               start=(s == 0 and e == 0),
                               stop=(s == 3 and e == 1), is_transpose=True)

                def trans_tail(q, psref):
                    dst = xe[:, :, 512 * q:512 * (q + 1)]
                    dst = dst.rearrange("p e (s c) -> p s e c", s=4)
                    return nc.vector.tensor_copy(out=dst, in_=psref[0])

                def theta_pe(e, c, psref):
                    sl = slice(CW * c, CW * (c + 1))
                    ps = pspool.tile([128, CW], F32, tag="cct", name="thps")
                    psref.append(ps)
                    mm(ps, wcembT_sb[:, 128 * e:128 * (e + 1)], oh_in[:, sl],
                       True, True)

                def theta_tail(e, c, psref):
                    sl = slice(CW * c, CW * (c + 1))
                    return nc.vector.tensor_mul(out=th[:, e, sl],
                                                in0=xe[:, e, sl], in1=psref[0])

                def xg_pe(j, c, psref):
                    sl = slice(CW * c, CW * (c + 1))
                    ps = pspool.tile([128, CW], F32, tag="xgps", name="xgps")
                    psref.append(ps)
                    for k in range(3):
                        if k < 2:
                            lhsT = wih_sb[:, (k * 8 + j) * 128:(k * 8 + j + 1) * 128]
                            rhs = th[:, k, sl]
                        else:
                            lhsT = wihct_sb[:, j * 128:(j + 1) * 128]
                            rhs = oh_in[:, sl]
                        mm(ps, lhsT, rhs, k == 0, k == 2)

                def xg_tail(j, c, psref):
                    sl = slice(CW * c, CW * (c + 1))
                    if (j + c) % 2 == 0:
                        return nc.scalar.copy(out=xg_t[:, j, sl],
                                              in_=psref[0])
                    return nc.vector.tensor_copy(out=xg_t[:, j, sl],
                                                 in_=psref[0])

                units = []

                def unit(pe, tail, *args):
                    psref = []
                    units.append((lambda: pe(*args, psref),
                                  lambda: tail(*args, psref)))

                for c in range(BT // CW):
                    unit(oh_pe, oh_tail, 0, c)
                    unit(oh_pe, oh_tail, 1, c)
                for q in range(NB128 // 4):
                    unit(trans_pe, trans_tail, q)
                for c in range(BT // CW):
                    for e in range(2):
                        unit(theta_pe, theta_tail, e, c)
                for j in range(8):
                    for c in range(BT // CW):
                        unit(xg_pe, xg_tail, j, c)
                return (oh_in, oh_out, th, xg_t), units

            # ---- phase-3 units for one 512-token group (16 steps) ----
            def p3_group_units(hb, oh_out, l, u0):
                t0 = l * BT
                g0 = 32 * u0
                tsl = slice(g0, g0 + 512)
                tho = spool.tile([128, 2, 512], BF16, tag="tho", bufs=2,
                                 name="tho")
                yss = [spool.tile([128, NUM_C], F32, tag="ys", bufs=2,
                                  name="ys") for _ in range(4)]
                units = []
                gate = {"n": 0}  # tho tails emitted so far for this group

                def tho_pe(e, psref):
                    ps = pspool.tile([128, 512], F32, tag="cct", name="cct3")
                    psref.append(ps)
                    mm(ps, wcembT_sb[:, 128 * e:128 * (e + 1)], oh_out[:, tsl],
                       True, True)

                def tho_tail(e, psref):
                    r = nc.vector.tensor_mul(out=tho[:, e, :],
                                             in0=hb[:, e, tsl],
                                             in1=psref[0])
                    gate["n"] += 1
                    if DBG and l == 0 and u0 == 0 and e == 1:
                        nc.sync.dma_start(out=dbg_tho[:, :], in_=tho)
                    return r

                def chunk_pe(sub, c, psref):
                    sl = slice(512 * c, 512 * (c + 1))
                    ps = pspool.tile([128, 512], F32, tag="psy", name="psy")
                    psref.append(ps)
                    for k in range(3):
                        if k < 2:
                            lhsT = tho[:, k, 128 * sub:128 * (sub + 1)]
                        else:
                            lhsT = oh_out[:, g0 + 128 * sub:g0 + 128 * (sub + 1)]
                        mm(ps, lhsT, (w0_sb, w1_sb, w2_sb)[k][:, sl],
                           k == 0, k == 2)

                def chunk_tail(sub, c, psref):
                    sl = slice(512 * c, 512 * (c + 1))
                    act = nc.scalar.activation(yss[sub][:, sl], psref[0],
                                               AF.Sigmoid)
                    if c == NUM_C // 512 - 1:
                        r0 = t0 + g0 + 128 * sub
                        nc.sync.dma_start(out=y[r0:r0 + 128, :], in_=yss[sub])
                    return act

                def unit(pe, tail, is_tho, *args):
                    psref = []
                    units.append((lambda: pe(*args, psref),
                                  lambda: tail(*args, psref), is_tho, gate))

                for e in range(2):
                    unit(tho_pe, tho_tail, True, e)
                for sub in range(4):
                    for c in range(NUM_C // 512):
                        unit(chunk_pe, chunk_tail, False, sub, c)
                return units

            # ---- prologue: phase-1 for block 0, fully drained ----
            cur, units = p1_alloc_units(0)
            for pe, tail in units:
                pe()
                tail()
            units = []
            tails_prev = []
            p3q = []
            hprev = None

            def pin_tail(tl):
                # anchor bulk tails behind the current step's chain so the
                # scheduler cannot hoist them between chain ops
                r = tl()
                if r is not None and state.get("anchor") is not None:
                    add_dep_helper(r.ins, state["anchor"].ins,
                                   reason="tail after chain")

            sg_cur = spool.tile([128, 10, 32], F32, tag="sg", name="sg0")

            for l in range(NBLK):
                oh_in, oh_out, th, xg_t = cur
                hb = bpool.tile([128, 2, BT], BF16, tag="hb", name="hb")
                for u in range(T_BLK):
                    t = l * T_BLK + u
                    # ---- LSTM chain ----
                    ps = pspool.tile([128, 8, 32], F32, tag="gates",
                                     name="gps")
                    mm(ps, ident_sb, xg_t[:, :, 32 * u:32 * (u + 1)],
                       True, t == 0)
                    if t > 0:
                        hbp, up = hprev
                        for k in range(2):
                            for j in range(8):
                                mm(ps[:, j, :],
                                   whh_sb[:, (k * 8 + j) * 128:(k * 8 + j + 1) * 128],
                                   hbp[:, k, 32 * up:32 * (up + 1)],
                                   False, k == 1)
                    sg_next = spool.tile([128, 10, 32], F32, tag="sg",
                                         name="sgn")
                    nc.scalar.activation(sg_cur[:, 0:8, :], ps, AF.Sigmoid)
                    # tg = 2*sigmoid(2g) - 1  (the 2x on g is in the weights)
                    nc.vector.tensor_scalar(
                        out=sg_cur[:, 6:8, :], in0=sg_cur[:, 6:8, :],
                        scalar1=2.0, scalar2=-1.0, op0=ALU.mult, op1=ALU.add)
                    if t == 0:
                        # c0 = i * tg (no carry yet)
                        nc.vector.tensor_mul(sg_next[:, 8:10, :],
                                             sg_cur[:, 0:2, :],
                                             sg_cur[:, 6:8, :])
                    else:
                        prod = spool.tile([128, 4, 32], F32, tag="prod",
                                          name="prod")
                        nc.vector.tensor_mul(prod, sg_cur[:, 0:4, :],
                                             sg_cur[:, 6:10, :])
                        nc.vector.tensor_add(sg_next[:, 8:10, :],
                                             prod[:, 0:2, :], prod[:, 2:4, :])
                    tct = spool.tile([128, 2, 32], BF16, tag="tct",
                                     name="tct")
                    nc.scalar.activation(tct, sg_next[:, 8:10, :], AF.Tanh)
                    hmul = nc.vector.tensor_mul(
                        hb[:, :, 32 * u:32 * (u + 1)], sg_cur[:, 4:6, :], tct)
                    state["anchor"] = hmul
                    if DBG and t < 4:
                        nc.sync.dma_start(out=dbg_sg[t], in_=sg_cur)
                    hprev = (hb, u)
                    sg_cur = sg_next

                    # ---- bulk production ----
                    if u == 0 and l + 1 < NBLK:
                        cur_next, units = p1_alloc_units(l + 1)
                    if u % 16 == 15:
                        p3q.extend(p3_group_units(hb, oh_out, l, u - 15))

                    # ---- bulk drain: tails from last step, then new PE ----
                    for tl in tails_prev:
                        pin_tail(tl)
                    tails_prev = []
                    if units and u >= 1:
                        pe, tail = units.pop(0)
                        pe()
                        tails_prev.append(tail)
                    np3 = 2 if len(p3q) >= 6 else (1 if p3q else 0)
                    tho_popped = False
                    for _ in range(np3):
                        if not p3q:
                            break
                        pe, tail, is_tho, g8 = p3q[0]
                        # at most one cct-psum (tho) unit per step: a third
                        # same-step cct alloc would alias a psum whose drain
                        # is emitted NEXT step -> PE-order deadlock with the
                        # frozen matmul chain.
                        if is_tho and tho_popped:
                            break
                        # chunk matmuls read both tho halves: only pop once
                        # both tho tails are EMITTED (else the read precedes
                        # the write in program order -> stale data race)
                        if not is_tho and g8["n"] < 2:
                            break
                        p3q.pop(0)
                        tho_popped |= is_tho
                        pe()
                        tails_prev.append(tail)
                while units:
                    for tl in tails_prev:
                        pin_tail(tl)
                    tails_prev = []
                    pe, tail = units.pop(0)
                    pe()
                    tails_prev.append(tail)
                if DBG and l == 0:
                    nc.sync.dma_start(out=dbg_xg[:, :, :], in_=xg_t)
                    nc.sync.dma_start(out=dbg_hb[:, :, :], in_=hb)
                    nc.sync.dma_start(out=dbg_oh[0], in_=oh_in)
                    nc.sync.dma_start(out=dbg_oh[1], in_=oh_out)
                if l + 1 < NBLK:
                    cur = cur_next
            while p3q or tails_prev:
                for tl in tails_prev:
                    pin_tail(tl)
                tails_prev = []
                if p3q:
                    pe, tail, is_tho, g8 = p3q[0]
                    if is_tho or g8["n"] >= 2:
                        p3q.pop(0)
                        pe()
                        tails_prev.append(tail)
    return nc


# ------------------------------------------------------------------
# host side
# ------------------------------------------------------------------

def _sel_patterns():
    pat = np.zeros((7, 128), np.float32)
    for which in range(2):
        o = 64 * which
        r = 3 * which
        pat[r + 0, o + 0:o + 16] = 1.0
        pat[r + 1, o + 16:o + 32] = 1.0
        pat[r + 2, o + 32:o + 64] = 1.0
        pat[6, o + 0:o + 16] = -np.arange(16)
        pat[6, o + 16:o + 32] = -np.arange(16)
        pat[6, o + 32:o + 64] = -np.arange(32)
    return pat.astype(BF)


def _tok(a):
    """[BLOC, S] -> [S*BLOC] in s-major token order."""
    return np.ascontiguousarray(a.T).reshape(-1)


def make_inputs(inputs, S=512, version=None):
    if version is None:
        version = KERNEL_VERSION
    """Build shared weight arrays + per-core in_maps from the full inputs."""
    NTOK = S * BLOC
    f32 = np.float32
    q = np.asarray(inputs["q"]).astype(np.int64)
    r = np.asarray(inputs["r"]).astype(np.int64)
    x = (q + NUM_C * r).astype(np.int32)
    rg = np.asarray(inputs["rgaps"]).astype(np.int32)
    sg = np.asarray(inputs["sgaps"]).astype(np.int32)
    pc = np.asarray(inputs["pcounts"]).astype(np.int32)
    srg = np.asarray(inputs["shft_rgaps"]).astype(np.int32)
    ssg = np.asarray(inputs["shft_sgaps"]).astype(np.int32)
    spc = np.asarray(inputs["shft_pcounts"]).astype(np.int32)
    E = np.asarray(inputs["E_inter"], f32)
    W_cemb = np.asarray(inputs["W_cemb"], f32)
    W_ih = np.asarray(inputs["W_ih"], f32)
    W_hh = np.asarray(inputs["W_hh"], f32)
    b = (np.asarray(inputs["b_ih"], f32) + np.asarray(inputs["b_hh"], f32))
    W_out = np.asarray(inputs["W_out"], f32)

    # gate reorder i,f,g,o -> i,f,o,g
    perm = np.r_[0:512, 768:1024, 512:768]
    Wih_p = W_ih[perm].copy()
    Whh_p = W_hh[perm].copy()
    bias_p = b[perm].copy()

    if version >= 3:
        # tanh(g) = 2*sigmoid(2g) - 1: fold the 2x into the g-gate rows so a
        # single sigmoid covers all 8 gate groups on device.
        Wih_p[768:1024] *= 2.0
        Whh_p[768:1024] *= 2.0
        bias_p[768:1024] *= 2.0

    def kmaj(A):  # [1024, 256] -> [128 k, 16 (kappa,j), 128 m]
        return np.ascontiguousarray(
            A.reshape(8, 128, 2, 128).transpose(3, 2, 0, 1)
        ).reshape(128, 16, 128).astype(BF)

    wihct_arr = np.ascontiguousarray(
        Wih_p[:, 256:320].reshape(8, 128, NT).transpose(2, 0, 1)).astype(f32)
    if version >= 3:
        # fold the LSTM bias into the rgap rows of wihct: exactly one of the
        # 16 rgap one-hot rows is 1 for every token, so adding the bias to
        # each of those rows adds it once per token.
        wihct_arr = wihct_arr.copy()
        wihct_arr[0:16] += bias_p.reshape(8, 128)[None, :, :]

    shared = {
        "selpat": _sel_patterns(),
        "emb": E.astype(BF),
        "wcembT": np.ascontiguousarray(W_cemb.T).astype(BF),
        "wih": kmaj(Wih_p[:, :256]),
        "wihct": wihct_arr.astype(BF),
        "whh": kmaj(Whh_p),
        "bias8": np.ascontiguousarray(bias_p.reshape(8, 128).T).astype(f32),
        "ident": np.eye(128).astype(BF),
        "woutT": np.ascontiguousarray(W_out.T).astype(BF),
    }

    in_maps = []
    ones = np.ones(NTOK, f32)
    for k in range(NCORES):
        bs = slice(BLOC * k, BLOC * (k + 1))
        tokv = _tok(x[bs])
        rows = np.stack([
            _tok(rg[bs]), _tok(sg[bs]), _tok(pc[bs]),
            _tok(srg[bs]), _tok(ssg[bs]), _tok(spc[bs]), ones]).astype(BF)
        m = dict(shared)
        if version >= 2:
            # gidx32[p, g] = token index of token g*128 + p
            m["gidx32"] = np.ascontiguousarray(
                tokv.reshape(NTOK // 128, 128).T.astype(np.int32))
        else:
            g16 = tokv.astype(np.int16).reshape(NTOK // 16, 16).T
            m["gidx"] = np.ascontiguousarray(np.tile(g16, (8, 1)))
        m["idxrows"] = np.ascontiguousarray(rows)
        in_maps.append(m)
    return in_maps


_NC_CACHE = {}
KERNEL_VERSION = int(os.environ.get("DKT_KERNEL_V", "3")) if True else 3


def kernel(**inputs):
    S = np.asarray(inputs["q"]).shape[1]
    key = S
    if key not in _NC_CACHE:
        T_BLK = 64 if S % 64 == 0 else S
        build = {1: build_nc, 2: build_nc_v2, 3: build_nc_v3}[KERNEL_VERSION]
        nc_new = build(S=S, T_BLK=T_BLK)
        split_excess_waits(nc_new)
        _NC_CACHE[key] = nc_new
    nc = _NC_CACHE[key]
    in_maps = make_inputs(inputs, S=S)
    res = run_bass_kernel_spmd(nc, in_maps, core_ids=list(range(NCORES)))
    outs = []
    for k in range(NCORES):
        yk = res.results[k]["y"]  # [NTOK, NUM_C] token order s-major
        outs.append(yk.reshape(S, BLOC, NUM_C).transpose(1, 0, 2))
    return np.ascontiguousarray(np.concatenate(outs, axis=0))

